# revision 54
# baseline (speedup 1.0000x reference)
"""Low-rank attention kernel for Trainium2, distributed over 8 NeuronCores.

Math (per batch b):
    u  = q @ Wu            [N, R]
    vp = k @ Wv            [N, R]
    S  = u @ vp.T / sqrt(R)
    out = softmax(S) @ v   [N, D]

Shapes: B=4, N=4096, D=1024, R=32.

Sharding: data-parallel over batch x row-halves -> 8 shards. Core c handles
batch b = c // 2, rows [h*2048, (h+1)*2048) with h = c % 2; each core sees
the full k/v of its batch.

Numerics: every tensor ships as an fp8e4 hi/lo pair quantized on the host
(hi = fp8(x), lo = fp8(x - hi) -> ~16-bit effective precision), so every
matmul runs in fp8 DoubleRow mode (0.5 cycles/row, two k-tiles contracted
per instruction - 4x f16 throughput per contracted key) while keeping the
end-to-end error at ~4e-3 vs the 2e-2 gate:

  projections  u/vp = 3 DoubleRow products per d-pair:
               wh.xh + wl.xh + wh.xl        (wl.xl dropped, ~0.06%)
               Wu/Wv are pre-scaled x64 on the host so their hi/lo planes
               sit in fp8e4's normal range (at natural scale they fall into
               subnormals and the scores degrade by ~2%).
  scores       f16 matmuls, K = R+1: uT/vpT carry an extra constant row
               (uT[32,:]=64, vpT[32,:]=-ln16*64/RSCALE) so exp comes out
               pre-scaled by 1/16 and fp8 attention weights max out at ~31
               (fp8e4 top is 240). The constant-row rounding is a uniform
               score shift and cancels exactly in the softmax.
  exp          ex16 = Exp(ps * RSCALE/4096) on ScalarE (f16)
               eh = fp8(ex16) on gpsimd (Pool), el = fp8(ex16 - eh) on DVE
  AV           acc[a] += eh(x)vh + eh(x)vl + el(x)vh per m-pair, both
               m-tiles in one DoubleRow instr (el(x)vl dropped, ~0.1%).
               All three terms are required: dropping any of them on any
               static subset of key blocks puts the global max-err metric
               over the 2e-2 gate (softmax rows concentrate their mass).
  row sums     tiny f16 ones-matmuls on ex16 (consistent with eh+el).

Flash pipeline: 8 super-steps of 256 query rows; the two 128-row halves run
as separate 2-bank accumulator streams (A at slot g, B lagging LAG slots)
so PSUM fits 2x2 acc banks + 2 score banks + 1 proj + 1 sums. Both recips
are hoisted to the sums-complete point so the B normalize needs only its
accs and the sums bank recycles immediately.

DMA: the Tile scheduler reorders by readiness, but queue CAPACITY in the
first ~30us is the binding constraint: SP (pure DMA) carries the whole
kt/qt00 ladder in deadline order; Act (shared with the exp stream) carries
kt01 + qt01 + late v groups + qt h1; the gpsimd swdge queue carries v0-v11
(prologue + one group per flash slot via the inject map) and two of the
final output DMAs. u-chunk projections inject at the latest slot their
consumers allow (their qt tiles leave the critical window entirely).
"""

import numpy as np

B, N, D, R = 4, 4096, 1024, 32
NLOC = N // 2            # rows per core
RSCALE = float(1.0 / np.sqrt(np.float32(R)))
LN16 = float(np.log(16.0))

N_CHUNK = 256            # rows of scores computed per PSUM round
D_HALF = 512             # PSUM bank width in fp32
# m-pair dial: pairs in DROP_B skip the el(x)vh term, pairs in DROP_C skip
# the eh(x)vl term (error is measured end-to-end; gate is 2e-2)
DROP_B = frozenset()
DROP_C = frozenset()

LAST_RESULT = None       # test.py reads exec_time_ns etc. from here
LAST_NC = None           # built Bass module, for test.py's bench loop
LAST_IN_MAPS = None      # per-core input maps, for test.py's bench loop


def _build():
    from concourse import bacc, mybir
    from concourse.tile import TileContext

    f32 = mybir.dt.float32
    f16 = mybir.dt.float16
    f8 = mybir.dt.float8e4
    EXP = mybir.ActivationFunctionType.Exp
    DR = mybir.MatmulPerfMode.DoubleRow

    nc = bacc.Bacc("TRN2", target_bir_lowering=False)

    qh_d = nc.dram_tensor("qh", [D, NLOC], f8, kind="ExternalInput")
    ql_d = nc.dram_tensor("ql", [D, NLOC], f8, kind="ExternalInput")
    kh_d = nc.dram_tensor("kh", [D, N], f8, kind="ExternalInput")
    kl_d = nc.dram_tensor("kl", [D, N], f8, kind="ExternalInput")
    vh_d = nc.dram_tensor("vh", [N, D], f8, kind="ExternalInput")
    vl_d = nc.dram_tensor("vl", [N, D], f8, kind="ExternalInput")
    # packed projection weights: [D, 4R] = wvh | wvl | wuh | wul
    wall = nc.dram_tensor("wall", [D, 4 * R], f8, kind="ExternalInput")
    o = nc.dram_tensor("o", [NLOC, D], f16, kind="ExternalOutput")

    DT = D // 128         # 8 d-tiles
    NCH = NLOC // N_CHUNK  # 8 flash chunks
    MT = N // 128         # 32 m tiles
    NP = MT // 2          # 16 m-tile pairs per chunk
    GP = NCH * NP         # 128 global pairs
    VG = 16               # v row-groups of 256
    VPG = N // VG // 128  # 2 m-tiles per v group

    with TileContext(nc) as tc:
        with tc.tile_pool(name="singles", bufs=1) as singles, \
             tc.tile_pool(name="stream", bufs=12) as stream, \
             tc.tile_pool(name="vpool", bufs=2 * VG) as vpool, \
             tc.tile_pool(name="expp", bufs=8) as expp, \
             tc.tile_pool(name="ehp", bufs=8) as ehp, \
             tc.tile_pool(name="elp", bufs=8) as elp, \
             tc.tile_pool(name="outp", bufs=6) as outp, \
             tc.tile_pool(name="rpool", bufs=6) as rpool, \
             tc.tile_pool(name="pacc", bufs=4, space="PSUM") as pacc, \
             tc.tile_pool(name="pscore", bufs=2, space="PSUM") as pscore, \
             tc.tile_pool(name="pproj", bufs=1, space="PSUM") as pproj, \
             tc.tile_pool(name="psums", bufs=1, space="PSUM") as psums:

            # ---- constants / projection weights (one packed DMA) ----
            wall_sb = singles.tile([128, DT, 4 * R], f8, tag="wall")
            nc.sync.dma_start(out=wall_sb,
                              in_=wall.rearrange("(t p) r -> p t r", p=128))
            wvh_sb = wall_sb[:, :, 0:R]
            wvl_sb = wall_sb[:, :, R:2 * R]
            wuh_sb = wall_sb[:, :, 2 * R:3 * R]
            wul_sb = wall_sb[:, :, 3 * R:4 * R]
            ones = singles.tile([128, 2], f16, tag="ones")
            nc.vector.memset(ones, 1.0)

            # uT/vpT carry an extra constant row 32: uT[32,:]=1, vpT[32,:]=
            # -ln16*sqrt(R), so the scores matmul yields u.vp - ln16*sqrt(R)
            # and the exp comes out pre-scaled by 1/16 (fp8 range headroom).
            # Constant-row f16 rounding is a uniform score shift -> cancels
            # exactly in the softmax.
            uT = singles.tile([R + 1, NLOC], f16, tag="uT")
            vpT = singles.tile([R + 1, N], f16, tag="vpT")
            nc.vector.memset(uT[R:R + 1, :], 64.0)
            nc.vector.memset(vpT[R:R + 1, :], -LN16 * 64.0 / RSCALE)

            # ---- DMA issue order = approximate arrival order ----
            # kT/qT loaded as [128, 8, 512] column-halves (one descriptor per
            # 512-col half across all 8 d-tiles: few, fat DMAs -> the SP
            # queue isn't descriptor-issue-bound). v groups interleaved in
            # the order the flash loop consumes them; qT h1 last (needed
            # from chunk 4, ~150us in).
            kt_tiles = {}
            qt_tiles = {}

            def _stream_pair(pfx, name, tiles_map, key):
                if key not in tiles_map:
                    hi = stream.tile([128, DT, 512], f8, tag="stream",
                                     name=f"{pfx}h{name}")
                    lo = stream.tile([128, DT, 512], f8, tag="stream",
                                     name=f"{pfx}l{name}")
                    tiles_map[key] = (hi, lo)
                return tiles_map[key]

            def _load_cols(tile, dram, col, parts, eng):
                dt2 = DT // parts
                for s in range(parts):
                    eng.dma_start(
                        out=tile[:, s * dt2:(s + 1) * dt2, :],
                        in_=dram[s * dt2 * 128:(s + 1) * dt2 * 128,
                                 col:col + 512].rearrange(
                            "(t p) c -> p t c", p=128))

            def load_kt(qtr, c2, which, parts=1, eng=None):
                hi, lo = _stream_pair("kt", f"{qtr}_{c2}", kt_tiles, (qtr, c2))
                dram = kh_d if which == "h" else kl_d
                _load_cols(hi if which == "h" else lo, dram,
                           qtr * 1024 + c2 * 512, parts, eng or nc.sync)

            def load_qt(h, c2, which, parts=1, eng=None):
                hi, lo = _stream_pair("qt", f"{h}_{c2}", qt_tiles, (h, c2))
                dram = qh_d if which == "h" else ql_d
                _load_cols(hi if which == "h" else lo, dram,
                           h * 1024 + c2 * 512, parts, eng or nc.sync)

            vh_sb = [None] * VG
            vl_sb = [None] * VG

            def load_v(g, which, eng=None):
                # vl is skipped for DROP_C pairs (their eh(x)vl instr never
                # runs).
                if which == "l" and g in DROP_C:
                    return
                eng = eng or nc.sync
                rows = VPG * 128
                dram = vh_d if which == "h" else vl_d
                vt = vpool.tile([128, VPG, D], f8, tag="v",
                                name=f"v{which}{g}")
                eng.dma_start(
                    out=vt, in_=dram[g * rows:(g + 1) * rows, :].rearrange(
                        "(t p) d -> p t d", p=128))
                if which == "h":
                    vh_sb[g] = vt
                else:
                    vl_sb[g] = vt

            # Deadline-ordered two-queue schedule: SP and Act run at the
            # same queue rate, so pair each SP item with an Act item of the
            # same deadline. qt01 (u_chunk(1)) and qt h1 are consumed from
            # pair ~26 / chunk 4 - they leave the critical window entirely.
            SYNC, ACT = nc.sync, nc.scalar
            # SP: pure-DMA queue carries the whole kt ladder + qt00 in
            # deadline order. Act (shared with exp work): kt01 (so the
            # prologue's two vp halves load in parallel), then qt01, the
            # late v groups and qt h1. Pool (gpsimd swdge): v0-v11, one
            # group per flash slot via the inject map.
            load_kt(0, 0, "h", parts=2, eng=SYNC)
            load_kt(0, 0, "l", eng=SYNC)
            load_qt(0, 0, "h", eng=SYNC)
            load_qt(0, 0, "l", eng=SYNC)
            load_kt(0, 1, "h", parts=2, eng=ACT)
            load_kt(0, 1, "l", eng=ACT)
            load_v(0, "h", eng=nc.gpsimd)
            load_v(0, "l", eng=nc.gpsimd)
            load_v(1, "h", eng=nc.gpsimd)
            load_v(1, "l", eng=nc.gpsimd)
            load_v(2, "h", eng=nc.gpsimd)
            load_v(2, "l", eng=nc.gpsimd)
            load_kt(1, 0, "h", eng=SYNC)
            load_kt(1, 0, "l", eng=SYNC)
            load_kt(1, 1, "h", eng=SYNC)
            load_kt(1, 1, "l", eng=SYNC)
            load_qt(0, 1, "h", eng=ACT)
            load_qt(0, 1, "l", eng=ACT)
            load_kt(2, 0, "h", eng=SYNC)
            load_kt(2, 0, "l", eng=SYNC)
            load_kt(2, 1, "h", eng=SYNC)
            load_kt(2, 1, "l", eng=SYNC)
            load_v(12, "h", eng=ACT)
            load_v(12, "l", eng=ACT)
            load_kt(3, 0, "h", eng=SYNC)
            load_kt(3, 0, "l", eng=SYNC)
            load_v(13, "h", eng=ACT)
            load_v(13, "l", eng=ACT)
            load_kt(3, 1, "h", eng=SYNC)
            load_kt(3, 1, "l", eng=SYNC)
            load_v(14, "h", eng=ACT)
            load_v(14, "l", eng=ACT)
            load_v(15, "h", eng=ACT)
            load_v(15, "l", eng=ACT)
            load_qt(1, 0, "h", eng=SYNC)
            load_qt(1, 0, "l", eng=ACT)
            load_qt(1, 1, "h", eng=SYNC)
            load_qt(1, 1, "l", eng=ACT)

            # dummy exp after the DMA issues (so the Activation engine's
            # descriptor issues aren't stuck behind the ones-memset dep):
            # forces the ScalarE act-func table DMA (~2.7us) to happen under
            # the input-DMA shadow, not on the first real exp
            warm = singles.tile([128, 2], f16, tag="warm")
            nc.scalar.activation(out=warm, in_=ones, func=EXP, scale=1.0)

            # ---- projection emitters (PE accum + DVE copy out of PSUM) ----
            # fp8 DoubleRow 3-product per d-pair: wh.xh + wh.xl + wl.xh
            # (dropped wl.xl ~ 0.06% relative on u/vp - invisible in scores).
            def proj_512(w_hi, w_lo, tiles, key, out_ap, name,
                         pair_major=False):
                x_hi, x_lo = tiles[key]
                pp = pproj.tile([R, 512], f32, tag="proj", name=name)
                DP = DT // 2
                if pair_major:
                    order = [(t, w, x) for t in range(DP)
                             for w, x in ((w_hi, x_hi), (w_lo, x_hi),
                                          (w_hi, x_lo))]
                else:
                    order = [(t, w, x)
                             for w, x in ((w_hi, x_hi), (w_lo, x_hi),
                                          (w_hi, x_lo))
                             for t in range(DP)]
                for i, (t, wt, xt) in enumerate(order):
                    sl = slice(2 * t, 2 * t + 2)
                    nc.tensor.matmul(pp, lhsT=wt[:, sl, :],
                                     rhs=xt[:, sl, :],
                                     start=(i == 0),
                                     stop=(i == len(order) - 1),
                                     skip_group_check=True,
                                     perf_mode=DR)
                with tc.high_priority():
                    nc.vector.tensor_copy(out=out_ap, in_=pp)

            def u_chunk(c):
                h, c2 = c // 2, c % 2
                proj_512(wuh_sb, wul_sb, qt_tiles, (h, c2),
                         uT[0:R, c * 512:(c + 1) * 512], f"pu{c}",
                         pair_major=False)

            def vp_half(qtr, c2):
                off = qtr * 1024 + c2 * 512
                proj_512(wvh_sb, wvl_sb, kt_tiles, (qtr, c2),
                         vpT[0:R, off:off + 512], f"pv{qtr}_{c2}",
                         pair_major=False)

            def vp_quarter(qtr):
                vp_half(qtr, 0)
                vp_half(qtr, 1)


            # ---- continuous flash pipeline over 128 global pairs ----
            # inject: global pair index -> thunk emitted before that pair's
            # scores are issued (slots projection work into the in-order PE
            # stream exactly where its inputs have arrived).
            def pool_v(g):
                load_v(g, "h", eng=nc.gpsimd)
                load_v(g, "l", eng=nc.gpsimd)

            inject = {2: lambda: vp_half(0, 1),
                      4: lambda: vp_half(1, 0),
                      5: lambda: pool_v(3),
                      6: lambda: (vp_half(1, 1), pool_v(4)),
                      7: lambda: pool_v(5),
                      8: lambda: (vp_half(2, 0), pool_v(6)),
                      9: lambda: pool_v(7),
                      10: lambda: (vp_half(2, 1), pool_v(8)),
                      11: lambda: pool_v(9),
                      12: lambda: (vp_half(3, 0), pool_v(10)),
                      13: lambda: pool_v(11),
                      14: lambda: vp_half(3, 1),
                      31: lambda: u_chunk(1),
                      56: lambda: u_chunk(2),
                      88: lambda: u_chunk(3)}

            def scores_exp(g):
                if g in inject:
                    inject[g]()
                ch = g // NP
                ps = pscore.tile([128, 2, N_CHUNK], f32, tag="scores",
                                 name=f"ps{g}")
                for i in range(2):
                    mt = 2 * (g % NP) + i
                    nc.tensor.matmul(
                        ps[:, i, :],
                        lhsT=vpT[:, mt * 128:(mt + 1) * 128],
                        rhs=uT[:, ch * N_CHUNK:(ch + 1) * N_CHUNK],
                        start=(i == 0), stop=(i == 1),
                        skip_group_check=True)
                ex16 = expp.tile([128, 2, N_CHUNK], f16, tag="ex",
                                 name=f"ex{g}")
                nc.scalar.activation(out=ex16, in_=ps, func=EXP,
                                     scale=RSCALE / 4096.0)
                eh = ehp.tile([128, 2, N_CHUNK], f8, tag="eh", name=f"eh{g}")
                nc.gpsimd.tensor_copy(out=eh, in_=ex16)
                el = None
                if (g % NP) not in DROP_B:
                    el = elp.tile([128, 2, N_CHUNK], f8, tag="el",
                                  name=f"el{g}")
                    nc.vector.scalar_tensor_tensor(
                        out=el, in0=ex16, scalar=1.0, in1=eh,
                        op0=mybir.AluOpType.mult, op1=mybir.AluOpType.subtract)
                return ex16, eh, el

            # ---- two-stream flash pipeline ----
            # The chunk's two 128-row halves (j=0 "A", j=1 "B") run as
            # separate 2-bank accumulator streams with the B stream LAG
            # slots behind. During the input-DMA window the B stream gives
            # the PE runnable work (its v groups arrived LAG slots ago) that
            # fills what would otherwise be arrival stalls; B work is emitted
            # FIRST in each slot so it sits ahead of any stalling scores/proj
            # instr in the in-order PE stream.
            LAG = 4

            def normalize_half(s, j, accs2, rcj, fine=False):
                # [128,512] muls split DVE/ScalarE; the two half-DMAs issue
                # from different HWDGE queues so output descriptors don't
                # serialize on one engine at the kernel tail. The kernel's
                # last normalizes go in d-quarters so the final o-DMAs start
                # sooner.
                ob = outp.tile([128, D], f16, tag="ob", name=f"ob{s}_{j}")
                row = s * N_CHUNK + j * 128
                if fine:
                    Q = D_HALF // 2
                    for qi in range(2):
                        sl = slice(qi * Q, (qi + 1) * Q)
                        nc.vector.tensor_scalar_mul(ob[:, sl], accs2[0][:, sl],
                                                    rcj)
                        eng = nc.sync if qi == 0 else nc.gpsimd
                        eng.dma_start(out=o[row:row + 128, sl],
                                      in_=ob[:, sl])
                    for qi in range(2):
                        sl = slice(D_HALF + qi * Q, D_HALF + (qi + 1) * Q)
                        nc.scalar.mul(ob[:, sl],
                                      accs2[1][:, sl.start - D_HALF:
                                               sl.stop - D_HALF], rcj)
                        eng = nc.scalar if qi == 0 else nc.gpsimd
                        eng.dma_start(out=o[row:row + 128, sl],
                                      in_=ob[:, sl])
                    return
                nc.vector.tensor_scalar_mul(ob[:, 0:D_HALF], accs2[0], rcj)
                nc.sync.dma_start(out=o[row:row + 128, 0:D_HALF],
                                  in_=ob[:, 0:D_HALF])
                nc.scalar.mul(ob[:, D_HALF:D], accs2[1], rcj)
                nc.scalar.dma_start(out=o[row:row + 128, D_HALF:D],
                                    in_=ob[:, D_HALF:D])

            def av_pair(accs2, ex3, p, j, is_first, is_last):
                # AV in fp8 DoubleRow: acc[a] += eh(x)vh (main) + eh(x)vl
                # (v residual) + el(x)vh (exp residual); both m-tiles of the
                # pair contract in one instr. DROP pairs skip residual terms.
                ex16, eh, el = ex3
                seq = [(eh, False)]
                if p not in DROP_C:
                    seq.append((eh, True))
                if p not in DROP_B:
                    seq.append((el, False))
                for idx, (lhs, use_vl) in enumerate(seq):
                    last_kind = idx == len(seq) - 1
                    rhs_t = vl_sb[p] if use_vl else vh_sb[p]
                    for a in (0, 1):
                        nc.tensor.matmul(
                            accs2[a],
                            lhsT=lhs[:, :, j * 128:(j + 1) * 128],
                            rhs=rhs_t[:, :, a * D_HALF:(a + 1) * D_HALF],
                            start=(is_first and idx == 0),
                            stop=(is_last and last_kind),
                            perf_mode=DR)

            # prologue: only the projections scores pair 0/1 need; the
            # second vp half of quarter 0 is injected at slot 2
            vp_half(0, 0)
            u_chunk(0)
            ex_q = [scores_exp(0), scores_exp(1)]
            exd = {}
            rc_store = {}
            accA = accB = None
            sums = None
            for slot in range(GP + LAG):
                bslot = slot - LAG
                if bslot >= 0:
                    sb, pb = bslot // NP, bslot % NP
                    if pb == 0:
                        accB = [pacc.tile([128, D_HALF], f32, tag="acc",
                                          name=f"accB{sb}_{a}")
                                for a in (0, 1)]
                    av_pair(accB, exd.pop(bslot), pb, 1,
                            pb == 0, pb == NP - 1)
                    if pb == NP - 1:
                        normalize_half(sb, 1, accB, rc_store.pop(sb),
                                       fine=(sb == NCH - 1))
                if slot >= GP:
                    continue
                s, p = slot // NP, slot % NP
                if p == 0:
                    accA = [pacc.tile([128, D_HALF], f32, tag="acc",
                                      name=f"accA{s}_{a}") for a in (0, 1)]
                    sums_t = psums.tile([128, 4], f32, tag="sums",
                                        name=f"sum{s}")
                    sums = [sums_t[:, 0:2], sums_t[:, 2:4]]
                ex3 = ex_q.pop(0)
                exd[slot] = ex3
                if slot + 2 < GP:
                    ex_q.append(scores_exp(slot + 2))
                ex16 = ex3[0]
                # row sums from the f16 exp (consistent with eh+el to ~0.1%)
                for i in range(2):
                    for j in range(2):
                        nc.tensor.matmul(
                            sums[j],
                            lhsT=ex16[:, i, j * 128:(j + 1) * 128], rhs=ones,
                            start=(p == 0 and i == 0 and j == 0),
                            stop=(p == NP - 1 and i == 1),
                            skip_group_check=True)
                av_pair(accA, ex3, p, 0, p == 0, p == NP - 1)
                if p == NP - 1:
                    # sums for BOTH halves are complete here (A cadence):
                    # hoist both recips so the B-stream normalize only
                    # needs its accs, and the sums bank can be reused by
                    # the next chunk immediately.
                    rcA = rpool.tile([128, 1], f32, tag="rc", name=f"rcA{s}")
                    nc.vector.reciprocal(rcA, sums[0][:, 0:1])
                    rcB = rpool.tile([128, 1], f32, tag="rc", name=f"rcB{s}")
                    nc.vector.reciprocal(rcB, sums[1][:, 0:1])
                    rc_store[s] = rcB
                    normalize_half(s, 0, accA, rcA)

    nc.finalize()
    return nc


def _hi_lo(a32):
    import ml_dtypes
    npf8 = ml_dtypes.float8_e4m3
    hi = a32.astype(npf8)
    lo = (a32 - hi.astype(np.float32)).astype(npf8)
    return np.ascontiguousarray(hi), np.ascontiguousarray(lo)


def _make_in_maps(q, k, v, Wu, Wv):
    kThl = [_hi_lo(np.ascontiguousarray(k[b].T).astype(np.float32))
            for b in range(B)]
    vhl = [_hi_lo(np.ascontiguousarray(v[b]).astype(np.float32))
           for b in range(B)]
    wuh, wul = _hi_lo(np.ascontiguousarray(Wu).astype(np.float32) * 64.0)
    wvh, wvl = _hi_lo(np.ascontiguousarray(Wv).astype(np.float32) * 64.0)
    wall = np.ascontiguousarray(np.concatenate([wvh, wvl, wuh, wul], axis=1))
    in_maps = []
    for core in range(8):
        b, h = core // 2, core % 2
        qh, ql = _hi_lo(np.ascontiguousarray(
            q[b].T[:, h * NLOC:(h + 1) * NLOC]).astype(np.float32))
        in_maps.append({
            "qh": qh, "ql": ql,
            "kh": kThl[b][0], "kl": kThl[b][1],
            "vh": vhl[b][0], "vl": vhl[b][1],
            "wall": wall,
        })
    return in_maps


def kernel(q, k, v, Wu, Wv):
    global LAST_RESULT, LAST_NC, LAST_IN_MAPS
    from concourse import bass_utils

    nc = _build()
    LAST_NC = nc

    in_maps = _make_in_maps(q, k, v, Wu, Wv)
    LAST_IN_MAPS = in_maps

    res = bass_utils.run_bass_kernel_spmd(nc, in_maps, core_ids=list(range(8)))
    LAST_RESULT = res

    out = np.empty((B, N, D), dtype=np.float32)
    for core in range(8):
        b, h = core // 2, core % 2
        out[b, h * NLOC:(h + 1) * NLOC, :] = \
            res.results[core]["o"].astype(np.float32)
    return out
